# revision 29
# baseline (speedup 1.0000x reference)
"""Causal single-head attention on 8 TRN2 NeuronCores.

Problem (hardcoded): x [4, 2048, 1024] f32; Wk, Wq, Wv [1024, 1024] f32.
  q = x @ Wk.T ; k = x @ Wq.T ; v = x @ Wv.T        (note ref's q/k weight swap)
  out = softmax(mask(q @ k.T) / sqrt(1024)) @ v

Sharding: 2 cores per batch, query-parallel.  Queries are carved into
128-query blocks; core h of a batch owns global blocks (0,3,4,7,8,11,
12,15) (h=0) or (1,2,5,6,9,10,13,14) (h=1), processed as query tiles
T0..T7 sorted by ascending context with uniform key-chunk budgets
2(i+1) — both cores' block contexts fit these budgets almost exactly,
so block-causal score/AV work is 72 chunk-slots per core vs 96 for a
2x512 split.  Every core runs the identical program (true SPMD);
causality and padding are encoded in per-core additive mask inputs.

Because budgets ascend with column position, the tiles active at key
chunk k are exactly the suffix i >= k//2: one matmul per 512-column
group (A = T0..T3, B = T4..T7) covers them, accumulating right-aligned
PSUM column slices as short tiles retire.  Instruction count and
PSUM-bank usage match the coarse scheme while skipping masked work.

K/V projections are fully pair-split: core h projects K and V only for
its own key half [1024h : 1024h+1024), then the halves are exchanged
with pair AllGathers through DRAM bounce buffers (K first — scores need
it earlier).  Both ranks read back BOTH gathered regions (region r holds
group-rank r's half), landing K/V in canonical key order on both ranks.
A tiny dependency-free warmup collective absorbs the CC stream's
first-op setup cost.  Readbacks are split across the Sync and GpSimd
queues (descriptor issue is ~600 ns, serialized per queue).

On-chip layout is feature-major (all host-side transposes are free):
  xT/wT in, Q^T/K^T feature-major, V sequence-major.  Scores are
  computed as S^T[k, q] so softmax needs no on-chip transpose: exp via
  ACT (no max subtraction — scaled scores are ~N(0,1)), sum-of-exp via a
  ones-column matmul, AV accumulates out^T[e, q] with V stationary.  The
  per-query 1/sum is broadcast across partitions with a K=1 PE matmul
  and applied by DVE during the PSUM->SBUF output copy.  Output returns
  as out^T, transposed back on the host.  All matmuls bf16 with fp32
  PSUM accumulation.
"""

import functools

import ml_dtypes
import numpy as np

B = 4
S = 2048
D = 1024
P = 128
DCH = D // P            # 8 contraction chunks
HALF = S // 2           # own key half (pair-split projections)
NH = HALF // P          # 8 key slices per half
NKB = S // P            # 16 key chunks total
NEG = np.float32(-30000.0)

# global 128-query block ids per (h, tile), ascending context; tile i has
# key-chunk budget 2(i+1), which both cores' block contexts fit
_QBLOCKS = ((0, 3, 4, 7, 8, 11, 12, 15), (1, 2, 5, 6, 9, 10, 13, 14))
# active-suffix widths per key chunk: group A (tiles 0-3), group B (4-7)
_WA_H = [(4 - k // 2) * P for k in range(8)]
_WB_H = [(4 - max(0, k // 2 - 4)) * P for k in range(16)]

_BF16 = ml_dtypes.bfloat16


@functools.lru_cache(maxsize=1)
def _build_nc():
    import concourse.bass as bass  # noqa: F401  (registers engines)
    import concourse.mybir as mybir
    from concourse import bacc, tile

    bf16 = mybir.dt.bfloat16
    f32 = mybir.dt.float32
    add = mybir.AluOpType.add
    mult = mybir.AluOpType.mult
    Exp = mybir.ActivationFunctionType.Exp
    PAIRS = [[2 * i, 2 * i + 1] for i in range(4)]

    nc = bacc.Bacc("TRN2", target_bir_lowering=False, debug=False, num_devices=8)

    xT = nc.declare_dram_parameter("xT", [D, HALF], bf16, isOutput=False)
    xqT = nc.declare_dram_parameter("xqT", [D, 8 * P], bf16, isOutput=False)
    wqT = nc.declare_dram_parameter("wqT", [D, D], bf16, isOutput=False)
    wkT = nc.declare_dram_parameter("wkT", [D, D], bf16, isOutput=False)
    wvT = nc.declare_dram_parameter("wvT", [D, D], bf16, isOutput=False)
    # per-chunk masks, left-packed to the active-suffix width: maskA row
    # block k covers group-A tiles k//2..3, maskB row block k covers
    # group-B tiles max(4,k//2)..7
    maskA = nc.declare_dram_parameter("maskA", [8 * P, 512], bf16,
                                      isOutput=False)
    maskB = nc.declare_dram_parameter("maskB", [16 * P, 512], bf16,
                                      isOutput=False)
    outT = nc.declare_dram_parameter("outT", [D, 8 * P], f32, isOutput=True)

    with tile.TileContext(nc) as tc:
        with (
            tc.tile_pool(name="kv", bufs=1) as kv,
            tc.tile_pool(name="dram", bufs=1, space="DRAM") as dram,
        ):
            # ---- persistent SBUF tensors --------------------------------
            kt_sb = [kv.tile([P, S], bf16, tag=f"kt{e}", name=f"kt{e}")
                     for e in range(DCH)]
            qt_sb = [kv.tile([P, 8 * P], bf16, tag=f"qt{e}", name=f"qt{e}")
                     for e in range(DCH)]
            v_sb = [kv.tile([P, D], bf16, tag=f"v{t}", name=f"v{t}")
                    for t in range(S // P)]
            ma_sb = [kv.tile([P, _WA_H[k]], bf16, tag=f"ma{k}", name=f"ma{k}")
                     for k in range(8)]
            mb_sb = [kv.tile([P, _WB_H[k]], bf16, tag=f"mb{k}", name=f"mb{k}")
                     for k in range(NKB)]
            ones_sb = kv.tile([P, 1], bf16, tag="ones", name="ones")
            nc.gpsimd.memset(ones_sb[:], 1.0)
            onesr = kv.tile([1, P], f32, tag="onesr", name="onesr")
            nc.gpsimd.memset(onesr[:], 1.0)
            # touch the Exp LUT once so the lazy activation-table load isn't
            # on the first score tile's critical path
            scr = kv.tile([P, 1], f32, tag="scr", name="scr")
            nc.scalar.activation(scr[:], ones_sb[:], Exp)

            # DRAM bounce buffers for the pair K/V exchange
            agin_k = dram.tile([D, HALF], bf16, name="agin_k")
            agout_k = dram.tile([2 * D, HALF], bf16, name="agout_k")
            agin_v = dram.tile([HALF, D], bf16, name="agin_v")
            agout_v = dram.tile([S, D], bf16, name="agout_v")
            agin_w = dram.tile([2, 64], bf16, name="agin_w")
            agout_w = dram.tile([4, 64], bf16, name="agout_w")

            # ---- phase 1: load inputs + QKV projections -----------------
            with (
                tc.tile_pool(name="inp", bufs=1) as inp,
                tc.tile_pool(name="pps", bufs=2, space="PSUM") as pps,
            ):
                x_sb = [inp.tile([P, HALF], bf16, tag=f"x{d}", name=f"x{d}")
                        for d in range(DCH)]
                xq_sb = [inp.tile([P, 8 * P], bf16, tag=f"xq{d}", name=f"xq{d}")
                         for d in range(DCH)]
                wq_sb = [inp.tile([P, D], bf16, tag=f"wq{d}", name=f"wq{d}")
                         for d in range(DCH)]
                wk_sb = [inp.tile([P, D], bf16, tag=f"wk{d}", name=f"wk{d}")
                         for d in range(DCH)]
                wv_sb = [inp.tile([P, D], bf16, tag=f"wv{d}", name=f"wv{d}")
                        for d in range(DCH)]
                # Whole-row loads in first-use order, spread across queues so
                # issue cost (~600 ns/descriptor, serialized per queue) does
                # not starve the K projection: x on Sync, wk on GpSimd, wv on
                # Scalar (ahead of the mask prefetch), wq/xq back on Sync.
                for d in range(DCH):
                    rows = slice(d * P, (d + 1) * P)
                    nc.sync.dma_start(out=x_sb[d][:], in_=xT[rows, :])
                    nc.gpsimd.dma_start(out=wk_sb[d][:], in_=wkT[rows, :])
                for d in range(DCH):
                    rows = slice(d * P, (d + 1) * P)
                    nc.scalar.dma_start(out=wv_sb[d][:], in_=wvT[rows, :])
                for d in range(DCH):
                    rows = slice(d * P, (d + 1) * P)
                    nc.sync.dma_start(out=wq_sb[d][:], in_=wqT[rows, :])
                    nc.sync.dma_start(out=xq_sb[d][:], in_=xqT[rows, :])
                # mask prefetch rides the Scalar queue behind wv; masks are
                # not needed until the score loop so this is fully hidden
                for k in range(8):
                    nc.scalar.dma_start(
                        out=ma_sb[k][:],
                        in_=maskA[k * P:(k + 1) * P, 0:_WA_H[k]])
                for k in range(NKB):
                    nc.scalar.dma_start(
                        out=mb_sb[k][:],
                        in_=maskB[k * P:(k + 1) * P, 0:_WB_H[k]])
                # Dependency-free warmup collective: the first op on the CC
                # stream pays a large one-time setup cost; burn it on a
                # throwaway exchange so the real K exchange moves data
                # sooner.  (Queued on GpSimd after the wk loads — the CC
                # instruction occupies its queue ~15 us for ring setup.)
                nc.gpsimd.collective_compute(
                    "AllGather", mybir.AluOpType.bypass,
                    replica_groups=PAIRS,
                    ins=[agin_w[:]], outs=[agout_w[:]],
                )

                # K^T for the own key half, staged into kt cols [0:HALF) and
                # bounced to DRAM per e-chunk so the exchange starts ASAP.
                for e in range(DCH):
                    esl = slice(e * P, (e + 1) * P)
                    pss = [pps.tile([P, 512], f32, tag=f"pj{i}",
                                    name=f"kps{e}_{i}") for i in range(2)]
                    for d in range(DCH):
                        for i in range(2):
                            nc.tensor.matmul(
                                pss[i][:], wk_sb[d][:, esl],
                                x_sb[d][:, i * 512:(i + 1) * 512],
                                start=(d == 0), stop=(d == DCH - 1),
                            )
                    for i in range(2):
                        nc.vector.tensor_copy(
                            kt_sb[e][:, i * 512:(i + 1) * 512], pss[i][:])
                    nc.sync.dma_start(out=agin_k[esl, :],
                                      in_=kt_sb[e][:, 0:HALF])
                nc.gpsimd.collective_compute(
                    "AllGather", mybir.AluOpType.bypass,
                    replica_groups=PAIRS,
                    ins=[agin_k[:]], outs=[agout_k[:]],
                )

                # V for the own key half (overlaps the K exchange)
                for t in range(NH):
                    tsl = slice(t * P, (t + 1) * P)
                    pss = [pps.tile([P, 512], f32, tag=f"pj{i}",
                                    name=f"vps{t}_{i}") for i in range(2)]
                    for d in range(DCH):
                        for i in range(2):
                            nc.tensor.matmul(
                                pss[i][:], x_sb[d][:, tsl],
                                wv_sb[d][:, i * 512:(i + 1) * 512],
                                start=(d == 0), stop=(d == DCH - 1),
                            )
                    for i in range(2):
                        nc.vector.tensor_copy(
                            v_sb[t][:, i * 512:(i + 1) * 512], pss[i][:])
                    nc.sync.dma_start(out=agin_v[tsl, :], in_=v_sb[t][:])
                nc.gpsimd.collective_compute(
                    "AllGather", mybir.AluOpType.bypass,
                    replica_groups=PAIRS,
                    ins=[agin_v[:]], outs=[agout_v[:]],
                )

                # K readback: BOTH regions (region r = group-rank r's half),
                # so the final key order is canonical on both ranks.  Split
                # across Sync and GpSimd to halve serialized issue.
                for e in range(DCH):
                    esl = slice(e * P, (e + 1) * P)
                    nc.sync.dma_start(out=kt_sb[e][:, 0:HALF],
                                      in_=agout_k[esl, :])
                    nc.gpsimd.dma_start(out=kt_sb[e][:, HALF:S],
                                        in_=agout_k[D + e * P:D + (e + 1) * P, :])

                # Q^T[e, q]: one wq weight tile drives both query halves.
                # Runs while the V exchange is in flight.
                for e in range(DCH):
                    esl = slice(e * P, (e + 1) * P)
                    pss = [pps.tile([P, 512], f32, tag=f"pj{qh}",
                                    name=f"qps{e}_{qh}") for qh in range(2)]
                    for d in range(DCH):
                        for qh in range(2):
                            nc.tensor.matmul(
                                pss[qh][:], wq_sb[d][:, esl],
                                xq_sb[d][:, qh * 512:(qh + 1) * 512],
                                start=(d == 0), stop=(d == DCH - 1),
                            )
                    for qh in range(2):
                        nc.vector.tensor_copy(
                            qt_sb[e][:, qh * 512:(qh + 1) * 512], pss[qh][:])

                # V readback (canonical order; slots 0..7 WAR on the bounce)
                for t in range(S // P):
                    eng = nc.sync if t % 2 == 0 else nc.gpsimd
                    eng.dma_start(out=v_sb[t][:],
                                  in_=agout_v[t * P:(t + 1) * P, :])

            # ---- phase 2: attention over 8 suffix-packed query tiles ----
            with (
                tc.tile_pool(name="pp", bufs=1) as pp,
                tc.tile_pool(name="ost", bufs=4) as ost,
                tc.tile_pool(name="msc", bufs=2) as msc,
                tc.tile_pool(name="scp", bufs=4, space="PSUM") as scp,
                tc.tile_pool(name="aps", bufs=2, space="PSUM") as aps,
            ):
                # Query tiles T0..T7 (128 queries each) sit column-adjacent
                # in qt/out, sorted by ascending context budget 2(i+1)
                # chunks.  At key chunk k the active tiles are exactly the
                # suffix i >= k//2, so one matmul per 512-column group
                # covers them: group A = T0..T3 (cols 0:512, k < 8), group
                # B = T4..T7 (cols 512:1024, all k).  PSUM accumulates
                # right-aligned slices as short tiles retire.
                ph_a, ph_b = [], []
                sum_a = aps.tile([1, 512], f32, tag="sum", name="sum_a")
                sum_b = aps.tile([1, 512], f32, tag="sum", name="sum_b")
                recb = {}

                def emit_sums(sps, ph, nk, off):
                    for kk in range(nk):
                        st = max(0, kk // 2 - off) * P
                        nc.tensor.matmul(sps[:, st:512], ones_sb[:, 0:1],
                                         ph[kk][:], start=(kk == 0),
                                         stop=(kk == nk - 1),
                                         skip_group_check=True)

                def emit_chain(tag, sps):
                    srow = msc.tile([1, 512], f32, tag="srow",
                                    name=f"srow{tag}")
                    nc.vector.tensor_copy(srow[:], sps[:])
                    bc = aps.tile([P, 512], f32, tag="sum", name=f"bc{tag}")
                    nc.tensor.matmul(bc[:], onesr[:, 0:P], srow[:],
                                     start=True, stop=True)
                    rb = msc.tile([P, 512], f32, tag="recb",
                                  name=f"recb{tag}")
                    nc.vector.reciprocal_approx_fast(out=rb[:], in_=bc[:])
                    recb[tag] = rb

                # Score loop.  Softmax sums are deferred a chunk so their
                # exp inputs are never on the PE critical path.
                for k in range(NKB):
                    ksl = slice(k * P, (k + 1) * P)
                    a = k // 2
                    wa = (4 - a) * P
                    bs = max(0, a - 4)
                    wb = (4 - bs) * P
                    pa = None
                    if k < 8:
                        pa = scp.tile([P, wa], f32, tag="sc", name=f"sca{k}")
                    pb = scp.tile([P, wb], f32, tag="sc", name=f"scb{k}")
                    for e in range(DCH):
                        if pa is not None:
                            nc.tensor.matmul(
                                pa[:], kt_sb[e][:, ksl],
                                qt_sb[e][:, a * P:512],
                                start=(e == 0), stop=(e == DCH - 1),
                                skip_group_check=True)
                        nc.tensor.matmul(
                            pb[:], kt_sb[e][:, ksl],
                            qt_sb[e][:, 512 + bs * P:1024],
                            start=(e == 0), stop=(e == DCH - 1),
                            skip_group_check=True)
                    if k == 8:
                        emit_sums(sum_a, ph_a, 8, 0)
                    if k == 9:
                        emit_chain("a", sum_a)
                    if pa is not None:
                        nc.vector.tensor_tensor(pa[:], pa[:],
                                                ma_sb[k][:], op=add)
                        ph = pp.tile([P, wa], bf16, tag=f"pa{k}",
                                     name=f"pa{k}")
                        nc.scalar.activation(ph[:], pa[:], Exp, scale=0.03125)
                        ph_a.append(ph)
                    nc.vector.tensor_tensor(pb[:], pb[:], mb_sb[k][:], op=add)
                    phx = pp.tile([P, wb], bf16, tag=f"pb{k}", name=f"pb{k}")
                    nc.scalar.activation(phx[:], pb[:], Exp, scale=0.03125)
                    ph_b.append(phx)

                emit_sums(sum_b, ph_b, NKB, 4)

                # AV.  Group A first (needs only V slices 0..7, which land
                # first from the exchange readback), then group B.
                def av_group(grp, ph, nk, off, tag):
                    csl = slice(0, 512) if grp == 0 else slice(512, 1024)
                    for e in range(DCH):
                        esl = slice(e * P, (e + 1) * P)
                        ps = aps.tile([P, 512], f32, tag="av",
                                      name=f"av{grp}_{e}")
                        for kk in range(nk):
                            st = max(0, kk // 2 - off) * P
                            nc.tensor.matmul(
                                ps[:, st:512], v_sb[kk][:, esl], ph[kk][:],
                                start=(kk == 0), stop=(kk == nk - 1),
                                skip_group_check=True)
                        if grp == 0 and e == 0:
                            # group-B recip chain: its bc matmul rides here
                            # so the DVE sum-row copy has already landed
                            emit_chain("b", sum_b)
                        ot = ost.tile([P, 512], f32, tag="ot",
                                      name=f"ot{grp}_{e}")
                        nc.vector.tensor_tensor(ot[:], ps[:], recb[tag][:],
                                                op=mult)
                        eng = nc.sync if e % 2 == 0 else nc.gpsimd
                        eng.dma_start(out=outT[esl, csl], in_=ot[:])

                av_group(0, ph_a, 8, 0, "a")
                av_group(1, ph_b, NKB, 4, "b")

    nc.compile()
    return nc


def _make_mask(q0: int, k: int) -> np.ndarray:
    kk = k * P + np.arange(P)[:, None]
    q = q0 + np.arange(P)[None, :]
    return np.where(kk <= q, np.float32(0.0), NEG).astype(_BF16)


def _build_masks(h: int):
    g = _QBLOCKS[h]
    ma = np.zeros((8 * P, 512), dtype=_BF16)
    for k in range(8):
        w = _WA_H[k]
        ma[k * P:(k + 1) * P, 0:w] = np.concatenate(
            [_make_mask(P * g[i], k) for i in range(k // 2, 4)], axis=1)
    mb = np.zeros((16 * P, 512), dtype=_BF16)
    for k in range(16):
        w = _WB_H[k]
        mb[k * P:(k + 1) * P, 0:w] = np.concatenate(
            [_make_mask(P * g[i], k) for i in range(max(4, k // 2), 8)],
            axis=1)
    return np.ascontiguousarray(ma), np.ascontiguousarray(mb)


def _in_maps(x, Wk, Wq, Wv):
    wq_t = np.ascontiguousarray(Wk.T.astype(_BF16))   # ref swap: q uses Wk
    wk_t = np.ascontiguousarray(Wq.T.astype(_BF16))
    wv_t = np.ascontiguousarray(Wv.T.astype(_BF16))
    masks = [_build_masks(0), _build_masks(1)]
    maps = []
    for c in range(8):
        b, h = divmod(c, 2)
        xb = x[b].astype(_BF16)
        # own key half only: this core projects K/V for keys
        # [HALF*h : HALF*(h+1)); the other half arrives via the exchange
        x_t = np.ascontiguousarray(xb[h * HALF:(h + 1) * HALF].T)
        xq_t = np.ascontiguousarray(np.concatenate(
            [xb[P * g:P * (g + 1)] for g in _QBLOCKS[h]], axis=0).T)
        maps.append({
            "xT": x_t,
            "xqT": xq_t,
            "wqT": wq_t,
            "wkT": wk_t,
            "wvT": wv_t,
            "maskA": masks[h][0],
            "maskB": masks[h][1],
        })
    return maps


def _assemble(results):
    out = np.empty((B, S, D), dtype=np.float32)
    for c, res in enumerate(results):
        b, h = divmod(c, 2)
        o = res["outT"]
        for i, g in enumerate(_QBLOCKS[h]):
            out[b, P * g:P * (g + 1)] = o[:, P * i:P * (i + 1)].T
    return out


def kernel(x, Wk, Wq, Wv, _trace=False):
    from concourse.bass_utils import run_bass_kernel_spmd

    nc = _build_nc()
    res = run_bass_kernel_spmd(nc, _in_maps(x, Wk, Wq, Wv), list(range(8)),
                               trace=_trace)
    out = _assemble(res.results)
    if _trace:
        return out, res
    return out


# revision 30
# speedup vs baseline: 1.1183x; 1.1183x over previous
"""Causal single-head attention on 8 TRN2 NeuronCores.

Problem (hardcoded): x [4, 2048, 1024] f32; Wk, Wq, Wv [1024, 1024] f32.
  q = x @ Wk.T ; k = x @ Wq.T ; v = x @ Wv.T        (note ref's q/k weight swap)
  out = softmax(mask(q @ k.T) / sqrt(1024)) @ v

Sharding: 2 cores per batch, query-parallel.  Queries are carved into
128-query blocks; core h of a batch owns global blocks (0,3,4,7,8,11,
12,15) (h=0) or (1,2,5,6,9,10,13,14) (h=1), processed as query tiles
T0..T7 sorted by ascending context with uniform key-chunk budgets
2(i+1) — both cores' block contexts fit these budgets almost exactly,
so block-causal score/AV work is 72 chunk-slots per core vs 96 for a
2x512 split.  Every core runs the identical program (true SPMD);
causality and padding are encoded in per-core additive mask inputs.

Because budgets ascend with column position, the tiles active at key
chunk k are exactly the suffix i >= k//2: one matmul per 512-column
group (A = T0..T3, B = T4..T7) covers them, accumulating right-aligned
PSUM column slices as short tiles retire.  Instruction count and
PSUM-bank usage match the coarse scheme while skipping masked work.

K/V projections are fully pair-split: core h projects K and V only for
its own key half [1024h : 1024h+1024), then the halves are exchanged
with pair AllGathers through DRAM bounce buffers (K first — scores need
it earlier).  Both ranks read back BOTH gathered regions (region r holds
group-rank r's half), landing K/V in canonical key order on both ranks.
A tiny dependency-free warmup collective absorbs the CC stream's
first-op setup cost.  Readbacks are split across the Sync and GpSimd
queues (descriptor issue is ~600 ns, serialized per queue).

On-chip layout is feature-major (all host-side transposes are free):
  xT/wT in, Q^T/K^T feature-major, V sequence-major.  Scores are
  computed as S^T[k, q] so softmax needs no on-chip transpose: exp via
  ACT (no max subtraction — scaled scores are ~N(0,1)), sum-of-exp via a
  ones-column matmul, AV accumulates out^T[e, q] with V stationary.  The
  per-query 1/sum is broadcast across partitions with a K=1 PE matmul
  and applied by DVE during the PSUM->SBUF output copy.  Output returns
  as out^T, transposed back on the host.  All matmuls bf16 with fp32
  PSUM accumulation.
"""

import functools

import ml_dtypes
import numpy as np

B = 4
S = 2048
D = 1024
P = 128
DCH = D // P            # 8 contraction chunks
HALF = S // 2           # own key half (pair-split projections)
NH = HALF // P          # 8 key slices per half
NKB = S // P            # 16 key chunks total
NEG = np.float32(-30000.0)

# global 128-query block ids per (h, tile), ascending context; tile i has
# key-chunk budget 2(i+1), which both cores' block contexts fit
_QBLOCKS = ((0, 3, 4, 7, 8, 11, 12, 15), (1, 2, 5, 6, 9, 10, 13, 14))
# active-suffix widths per key chunk: group A (tiles 0-3), group B (4-7)
_WA_H = [(4 - k // 2) * P for k in range(8)]
_WB_H = [(4 - max(0, k // 2 - 4)) * P for k in range(16)]

_BF16 = ml_dtypes.bfloat16


@functools.lru_cache(maxsize=1)
def _build_nc():
    import concourse.bass as bass  # noqa: F401  (registers engines)
    import concourse.mybir as mybir
    from concourse import bacc, tile

    bf16 = mybir.dt.bfloat16
    f32 = mybir.dt.float32
    add = mybir.AluOpType.add
    mult = mybir.AluOpType.mult
    Exp = mybir.ActivationFunctionType.Exp
    PAIRS = [[2 * i, 2 * i + 1] for i in range(4)]

    nc = bacc.Bacc("TRN2", target_bir_lowering=False, debug=False, num_devices=8)

    xT = nc.declare_dram_parameter("xT", [D, HALF], bf16, isOutput=False)
    xqT = nc.declare_dram_parameter("xqT", [D, 8 * P], bf16, isOutput=False)
    wqT = nc.declare_dram_parameter("wqT", [D, D], bf16, isOutput=False)
    wkT = nc.declare_dram_parameter("wkT", [D, D], bf16, isOutput=False)
    wvT = nc.declare_dram_parameter("wvT", [D, D], bf16, isOutput=False)
    # per-chunk masks, left-packed to the active-suffix width: maskA row
    # block k covers group-A tiles k//2..3, maskB row block k covers
    # group-B tiles max(4,k//2)..7
    maskA = nc.declare_dram_parameter("maskA", [8 * P, 512], bf16,
                                      isOutput=False)
    maskB = nc.declare_dram_parameter("maskB", [16 * P, 512], bf16,
                                      isOutput=False)
    outT = nc.declare_dram_parameter("outT", [D, 8 * P], f32, isOutput=True)

    with tile.TileContext(nc) as tc:
        with (
            tc.tile_pool(name="kv", bufs=1) as kv,
            tc.tile_pool(name="dram", bufs=1, space="DRAM") as dram,
        ):
            # ---- persistent SBUF tensors --------------------------------
            kt_sb = [kv.tile([P, S], bf16, tag=f"kt{e}", name=f"kt{e}")
                     for e in range(DCH)]
            qt_sb = [kv.tile([P, 8 * P], bf16, tag=f"qt{e}", name=f"qt{e}")
                     for e in range(DCH)]
            v_sb = [kv.tile([P, D], bf16, tag=f"v{t}", name=f"v{t}")
                    for t in range(S // P)]
            ma_sb = [kv.tile([P, _WA_H[k]], bf16, tag=f"ma{k}", name=f"ma{k}")
                     for k in range(8)]
            mb_sb = [kv.tile([P, _WB_H[k]], bf16, tag=f"mb{k}", name=f"mb{k}")
                     for k in range(NKB)]
            ones_sb = kv.tile([P, 1], bf16, tag="ones", name="ones")
            nc.gpsimd.memset(ones_sb[:], 1.0)
            onesr = kv.tile([1, P], f32, tag="onesr", name="onesr")
            nc.gpsimd.memset(onesr[:], 1.0)
            # touch the Exp LUT once so the lazy activation-table load isn't
            # on the first score tile's critical path
            scr = kv.tile([P, 1], f32, tag="scr", name="scr")
            nc.scalar.activation(scr[:], ones_sb[:], Exp)

            # DRAM bounce buffers for the pair K/V exchange
            agin_k = dram.tile([D, HALF], bf16, name="agin_k")
            agout_k = dram.tile([2 * D, HALF], bf16, name="agout_k")
            agin_v = dram.tile([HALF, D], bf16, name="agin_v")
            agout_v = dram.tile([S, D], bf16, name="agout_v")
            agin_w = dram.tile([2, 64], bf16, name="agin_w")
            agout_w = dram.tile([4, 64], bf16, name="agout_w")

            # ---- phase 1: load inputs + QKV projections -----------------
            with (
                tc.tile_pool(name="inp", bufs=1) as inp,
                tc.tile_pool(name="pps", bufs=2, space="PSUM") as pps,
            ):
                x_sb = [inp.tile([P, HALF], bf16, tag=f"x{d}", name=f"x{d}")
                        for d in range(DCH)]
                xq_sb = [inp.tile([P, 8 * P], bf16, tag=f"xq{d}", name=f"xq{d}")
                         for d in range(DCH)]
                wq_sb = [inp.tile([P, D], bf16, tag=f"wq{d}", name=f"wq{d}")
                         for d in range(DCH)]
                wk_sb = [inp.tile([P, D], bf16, tag=f"wk{d}", name=f"wk{d}")
                         for d in range(DCH)]
                wv_sb = [inp.tile([P, D], bf16, tag=f"wv{d}", name=f"wv{d}")
                        for d in range(DCH)]
                # Whole-row loads in first-use order, spread across queues so
                # issue cost (~600 ns/descriptor, serialized per queue) does
                # not starve the K projection: x on Sync, wk on GpSimd, wv on
                # Scalar (ahead of the mask prefetch), wq/xq back on Sync.
                for d in range(DCH):
                    rows = slice(d * P, (d + 1) * P)
                    nc.sync.dma_start(out=x_sb[d][:], in_=xT[rows, :])
                    nc.gpsimd.dma_start(out=wk_sb[d][:], in_=wkT[rows, :])
                for d in range(DCH):
                    rows = slice(d * P, (d + 1) * P)
                    nc.scalar.dma_start(out=wv_sb[d][:], in_=wvT[rows, :])
                for d in range(DCH):
                    rows = slice(d * P, (d + 1) * P)
                    nc.sync.dma_start(out=wq_sb[d][:], in_=wqT[rows, :])
                    nc.sync.dma_start(out=xq_sb[d][:], in_=xqT[rows, :])
                # mask prefetch rides the Scalar queue behind wv; masks are
                # not needed until the score loop so this is fully hidden
                for k in range(8):
                    nc.scalar.dma_start(
                        out=ma_sb[k][:],
                        in_=maskA[k * P:(k + 1) * P, 0:_WA_H[k]])
                for k in range(NKB):
                    nc.scalar.dma_start(
                        out=mb_sb[k][:],
                        in_=maskB[k * P:(k + 1) * P, 0:_WB_H[k]])
                # Dependency-free warmup collective: the first op on the CC
                # stream pays a large one-time setup cost; burn it on a
                # throwaway exchange so the real K exchange moves data
                # sooner.  (Queued on GpSimd after the wk loads — the CC
                # instruction occupies its queue ~15 us for ring setup.)
                nc.gpsimd.collective_compute(
                    "AllGather", mybir.AluOpType.bypass,
                    replica_groups=PAIRS,
                    ins=[agin_w[:]], outs=[agout_w[:]],
                )

                # K^T for the own key half, staged into kt cols [0:HALF) and
                # bounced to DRAM per e-chunk so the exchange starts ASAP.
                for e in range(DCH):
                    esl = slice(e * P, (e + 1) * P)
                    pss = [pps.tile([P, 512], f32, tag=f"pj{i}",
                                    name=f"kps{e}_{i}") for i in range(2)]
                    for d in range(DCH):
                        for i in range(2):
                            nc.tensor.matmul(
                                pss[i][:], wk_sb[d][:, esl],
                                x_sb[d][:, i * 512:(i + 1) * 512],
                                start=(d == 0), stop=(d == DCH - 1),
                            )
                    for i in range(2):
                        nc.vector.tensor_copy(
                            kt_sb[e][:, i * 512:(i + 1) * 512], pss[i][:])
                    nc.sync.dma_start(out=agin_k[esl, :],
                                      in_=kt_sb[e][:, 0:HALF])
                nc.gpsimd.collective_compute(
                    "AllGather", mybir.AluOpType.bypass,
                    replica_groups=PAIRS,
                    ins=[agin_k[:]], outs=[agout_k[:]],
                )

                # V for the own key half (overlaps the K exchange)
                for t in range(NH):
                    tsl = slice(t * P, (t + 1) * P)
                    pss = [pps.tile([P, 512], f32, tag=f"pj{i}",
                                    name=f"vps{t}_{i}") for i in range(2)]
                    for d in range(DCH):
                        for i in range(2):
                            nc.tensor.matmul(
                                pss[i][:], x_sb[d][:, tsl],
                                wv_sb[d][:, i * 512:(i + 1) * 512],
                                start=(d == 0), stop=(d == DCH - 1),
                            )
                    for i in range(2):
                        nc.vector.tensor_copy(
                            v_sb[t][:, i * 512:(i + 1) * 512], pss[i][:])
                    nc.sync.dma_start(out=agin_v[tsl, :], in_=v_sb[t][:])
                nc.gpsimd.collective_compute(
                    "AllGather", mybir.AluOpType.bypass,
                    replica_groups=PAIRS,
                    ins=[agin_v[:]], outs=[agout_v[:]],
                )

                # K readback: BOTH regions (region r = group-rank r's half),
                # so the final key order is canonical on both ranks.  All on
                # Sync: on GpSimd these DMAs queue behind the CC_V
                # instruction, whose variable retirement can starve score
                # chunks 8-15 of kt region 1 for tens of us.
                for e in range(DCH):
                    esl = slice(e * P, (e + 1) * P)
                    nc.sync.dma_start(out=kt_sb[e][:, 0:HALF],
                                      in_=agout_k[esl, :])
                    nc.sync.dma_start(out=kt_sb[e][:, HALF:S],
                                        in_=agout_k[D + e * P:D + (e + 1) * P, :])

                # Q^T[e, q]: one wq weight tile drives both query halves.
                # Runs while the V exchange is in flight.
                for e in range(DCH):
                    esl = slice(e * P, (e + 1) * P)
                    pss = [pps.tile([P, 512], f32, tag=f"pj{qh}",
                                    name=f"qps{e}_{qh}") for qh in range(2)]
                    for d in range(DCH):
                        for qh in range(2):
                            nc.tensor.matmul(
                                pss[qh][:], wq_sb[d][:, esl],
                                xq_sb[d][:, qh * 512:(qh + 1) * 512],
                                start=(d == 0), stop=(d == DCH - 1),
                            )
                    for qh in range(2):
                        nc.vector.tensor_copy(
                            qt_sb[e][:, qh * 512:(qh + 1) * 512], pss[qh][:])

                # V readback (canonical order; slots 0..7 WAR on the bounce)
                for t in range(S // P):
                    eng = nc.sync if t % 2 == 0 else nc.gpsimd
                    eng.dma_start(out=v_sb[t][:],
                                  in_=agout_v[t * P:(t + 1) * P, :])

            # ---- phase 2: attention over 8 suffix-packed query tiles ----
            with (
                tc.tile_pool(name="pp", bufs=1) as pp,
                tc.tile_pool(name="ost", bufs=4) as ost,
                tc.tile_pool(name="msc", bufs=2) as msc,
                tc.tile_pool(name="scp", bufs=4, space="PSUM") as scp,
                tc.tile_pool(name="aps", bufs=2, space="PSUM") as aps,
            ):
                # Query tiles T0..T7 (128 queries each) sit column-adjacent
                # in qt/out, sorted by ascending context budget 2(i+1)
                # chunks.  At key chunk k the active tiles are exactly the
                # suffix i >= k//2, so one matmul per 512-column group
                # covers them: group A = T0..T3 (cols 0:512, k < 8), group
                # B = T4..T7 (cols 512:1024, all k).  PSUM accumulates
                # right-aligned slices as short tiles retire.
                ph_a, ph_b = [], []
                sum_a = aps.tile([1, 512], f32, tag="sum", name="sum_a")
                sum_b = aps.tile([1, 512], f32, tag="sum", name="sum_b")
                recb = {}

                def emit_sums(sps, ph, nk, off):
                    for kk in range(nk):
                        st = max(0, kk // 2 - off) * P
                        nc.tensor.matmul(sps[:, st:512], ones_sb[:, 0:1],
                                         ph[kk][:], start=(kk == 0),
                                         stop=(kk == nk - 1),
                                         skip_group_check=True)

                def emit_chain(tag, sps):
                    srow = msc.tile([1, 512], f32, tag="srow",
                                    name=f"srow{tag}")
                    nc.vector.tensor_copy(srow[:], sps[:])
                    bc = aps.tile([P, 512], f32, tag="sum", name=f"bc{tag}")
                    nc.tensor.matmul(bc[:], onesr[:, 0:P], srow[:],
                                     start=True, stop=True)
                    rb = msc.tile([P, 512], f32, tag="recb",
                                  name=f"recb{tag}")
                    nc.vector.reciprocal_approx_fast(out=rb[:], in_=bc[:])
                    recb[tag] = rb

                # Score loop.  Softmax sums are deferred a chunk so their
                # exp inputs are never on the PE critical path.
                for k in range(NKB):
                    ksl = slice(k * P, (k + 1) * P)
                    a = k // 2
                    wa = (4 - a) * P
                    bs = max(0, a - 4)
                    wb = (4 - bs) * P
                    pa = None
                    if k < 8:
                        pa = scp.tile([P, wa], f32, tag="sc", name=f"sca{k}")
                    pb = scp.tile([P, wb], f32, tag="sc", name=f"scb{k}")
                    for e in range(DCH):
                        if pa is not None:
                            nc.tensor.matmul(
                                pa[:], kt_sb[e][:, ksl],
                                qt_sb[e][:, a * P:512],
                                start=(e == 0), stop=(e == DCH - 1),
                                skip_group_check=True)
                        nc.tensor.matmul(
                            pb[:], kt_sb[e][:, ksl],
                            qt_sb[e][:, 512 + bs * P:1024],
                            start=(e == 0), stop=(e == DCH - 1),
                            skip_group_check=True)
                    if k == 8:
                        emit_sums(sum_a, ph_a, 8, 0)
                    if k == 9:
                        emit_chain("a", sum_a)
                    if pa is not None:
                        nc.vector.tensor_tensor(pa[:], pa[:],
                                                ma_sb[k][:], op=add)
                        ph = pp.tile([P, wa], bf16, tag=f"pa{k}",
                                     name=f"pa{k}")
                        nc.scalar.activation(ph[:], pa[:], Exp, scale=0.03125)
                        ph_a.append(ph)
                    nc.vector.tensor_tensor(pb[:], pb[:], mb_sb[k][:], op=add)
                    phx = pp.tile([P, wb], bf16, tag=f"pb{k}", name=f"pb{k}")
                    nc.scalar.activation(phx[:], pb[:], Exp, scale=0.03125)
                    ph_b.append(phx)

                emit_sums(sum_b, ph_b, NKB, 4)

                # AV.  Group A first (needs only V slices 0..7, which land
                # first from the exchange readback), then group B.
                def av_group(grp, ph, nk, off, tag):
                    csl = slice(0, 512) if grp == 0 else slice(512, 1024)
                    for e in range(DCH):
                        esl = slice(e * P, (e + 1) * P)
                        ps = aps.tile([P, 512], f32, tag="av",
                                      name=f"av{grp}_{e}")
                        for kk in range(nk):
                            st = max(0, kk // 2 - off) * P
                            nc.tensor.matmul(
                                ps[:, st:512], v_sb[kk][:, esl], ph[kk][:],
                                start=(kk == 0), stop=(kk == nk - 1),
                                skip_group_check=True)
                        if grp == 0 and e == 0:
                            # group-B recip chain: its bc matmul rides here
                            # so the DVE sum-row copy has already landed
                            emit_chain("b", sum_b)
                        ot = ost.tile([P, 512], f32, tag="ot",
                                      name=f"ot{grp}_{e}")
                        nc.vector.tensor_tensor(ot[:], ps[:], recb[tag][:],
                                                op=mult)
                        eng = nc.sync if e % 2 == 0 else nc.gpsimd
                        eng.dma_start(out=outT[esl, csl], in_=ot[:])

                av_group(0, ph_a, 8, 0, "a")
                av_group(1, ph_b, NKB, 4, "b")

    nc.compile()
    return nc


def _make_mask(q0: int, k: int) -> np.ndarray:
    kk = k * P + np.arange(P)[:, None]
    q = q0 + np.arange(P)[None, :]
    return np.where(kk <= q, np.float32(0.0), NEG).astype(_BF16)


def _build_masks(h: int):
    g = _QBLOCKS[h]
    ma = np.zeros((8 * P, 512), dtype=_BF16)
    for k in range(8):
        w = _WA_H[k]
        ma[k * P:(k + 1) * P, 0:w] = np.concatenate(
            [_make_mask(P * g[i], k) for i in range(k // 2, 4)], axis=1)
    mb = np.zeros((16 * P, 512), dtype=_BF16)
    for k in range(16):
        w = _WB_H[k]
        mb[k * P:(k + 1) * P, 0:w] = np.concatenate(
            [_make_mask(P * g[i], k) for i in range(max(4, k // 2), 8)],
            axis=1)
    return np.ascontiguousarray(ma), np.ascontiguousarray(mb)


def _in_maps(x, Wk, Wq, Wv):
    wq_t = np.ascontiguousarray(Wk.T.astype(_BF16))   # ref swap: q uses Wk
    wk_t = np.ascontiguousarray(Wq.T.astype(_BF16))
    wv_t = np.ascontiguousarray(Wv.T.astype(_BF16))
    masks = [_build_masks(0), _build_masks(1)]
    maps = []
    for c in range(8):
        b, h = divmod(c, 2)
        xb = x[b].astype(_BF16)
        # own key half only: this core projects K/V for keys
        # [HALF*h : HALF*(h+1)); the other half arrives via the exchange
        x_t = np.ascontiguousarray(xb[h * HALF:(h + 1) * HALF].T)
        xq_t = np.ascontiguousarray(np.concatenate(
            [xb[P * g:P * (g + 1)] for g in _QBLOCKS[h]], axis=0).T)
        maps.append({
            "xT": x_t,
            "xqT": xq_t,
            "wqT": wq_t,
            "wkT": wk_t,
            "wvT": wv_t,
            "maskA": masks[h][0],
            "maskB": masks[h][1],
        })
    return maps


def _assemble(results):
    out = np.empty((B, S, D), dtype=np.float32)
    for c, res in enumerate(results):
        b, h = divmod(c, 2)
        o = res["outT"]
        for i, g in enumerate(_QBLOCKS[h]):
            out[b, P * g:P * (g + 1)] = o[:, P * i:P * (i + 1)].T
    return out


def kernel(x, Wk, Wq, Wv, _trace=False):
    from concourse.bass_utils import run_bass_kernel_spmd

    nc = _build_nc()
    res = run_bass_kernel_spmd(nc, _in_maps(x, Wk, Wq, Wv), list(range(8)),
                               trace=_trace)
    out = _assemble(res.results)
    if _trace:
        return out, res
    return out
